# revision 3
# baseline (speedup 1.0000x reference)
"""Expert-parallel Switch-Transformer MoE layer for 8 Trainium2 NeuronCores.

Strategy (SPMD, one program, per-core inputs):
  - Token-parallel router: core k routes tokens [k*2048,(k+1)*2048) in fp32
    (argmax must match the fp32 reference; bf16 flips ~31 tokens).
  - Block-triangular-matmul cumsum gives per-shard expert positions; an
    AllGather of per-shard expert counts gives global first-come positions.
  - Global slot = eid*C + pos; slots AllGathered; each core builds its own
    expert's slot->token table with a 256B-row DMA scatter.
  - Expert-parallel FFN (bf16): core k holds expert k's weights; tokens are
    fetched by transpose-dma_gather from a replicated bf16 copy of x.
  - Combine: per-chunk AllGather of expert outputs; each core gathers its own
    tokens' rows by slot, applies its locally-known fp32 gate, writes its
    token shard. Host concatenates the 8 shards.
"""
import sys

for _p in ("/opt/trn_rl_repo", "/root/.axon_site/_ro/trn_rl_repo"):
    if _p not in sys.path:
        sys.path.append(_p)

import numpy as np

import concourse.bacc as bacc
import concourse.bass as bass
import concourse.mybir as mybir
import concourse.tile as tile
from concourse import bass_utils

F32 = mybir.dt.float32
BF16 = mybir.dt.bfloat16
I16 = mybir.dt.int16
I32 = mybir.dt.int32
U32 = mybir.dt.uint32
Alu = mybir.AluOpType
Act = mybir.ActivationFunctionType
X = mybir.AxisListType.X

T, D, E, H = 16384, 1024, 8, 4096
NC = 8
TS = T // NC                # tokens per shard = 2048
C = T // E                  # expert capacity = 2048
NTT = TS // 128             # 16 token tiles per shard
CHUNK = 256                 # FFN token chunk
NCH = C // CHUNK            # 8 chunks
NJT = H // 128              # 32
SENT = T                    # sentinel row id -> zero pad row
RG = [list(range(NC))]

DEBUG_OUTPUTS = False


def build(debug_outputs=DEBUG_OUTPUTS):
    nc = bacc.Bacc("TRN2", target_bir_lowering=False, debug=False, num_devices=NC)

    # --------- per-core inputs (host pre-arranged, straight DMA loads) ---------
    xT = nc.dram_tensor("xT", [128, E, TS], F32, kind="ExternalInput")
    wr1 = nc.dram_tensor("wr1", [128, E, D], F32, kind="ExternalInput")
    wr2 = nc.dram_tensor("wr2", [128, E, E], F32, kind="ExternalInput")
    br1 = nc.dram_tensor("br1", [128, E], F32, kind="ExternalInput")
    br2 = nc.dram_tensor("br2", [1, E], F32, kind="ExternalInput")
    xbf = nc.dram_tensor("xbf", [T + 1, D], BF16, kind="ExternalInput")
    w1 = nc.dram_tensor("w1", [128, E, H], BF16, kind="ExternalInput")
    w2 = nc.dram_tensor("w2", [128, NJT, D], BF16, kind="ExternalInput")
    b1 = nc.dram_tensor("b1", [128, NJT], F32, kind="ExternalInput")
    b2 = nc.dram_tensor("b2", [1, D], F32, kind="ExternalInput")
    UT = nc.dram_tensor("UT", [128, 128], F32, kind="ExternalInput")
    IOTA8 = nc.dram_tensor("IOTA8", [128, E], F32, kind="ExternalInput")
    TOKID = nc.dram_tensor("TOKID", [128, 128], F32, kind="ExternalInput")
    MASK = nc.dram_tensor("MASK", [E, 128], F32, kind="ExternalInput")
    SHIFTK = nc.dram_tensor("SHIFTK", [128, 1], F32, kind="ExternalInput")

    out = nc.dram_tensor("out", [TS, D], F32, kind="ExternalOutput")
    if debug_outputs:
        dbg_slot = nc.dram_tensor("dbg_slot", [128, NTT], F32, kind="ExternalOutput")
        dbg_gate = nc.dram_tensor("dbg_gate", [128, NTT], F32, kind="ExternalOutput")
        dbg_eid = nc.dram_tensor("dbg_eid", [128, NTT], F32, kind="ExternalOutput")
        dbg_rec = nc.dram_tensor("dbg_rec", [128, NTT], F32, kind="ExternalOutput")
        dbg_cnt = nc.dram_tensor("dbg_cnt", [E, E], F32, kind="ExternalOutput")
        dbg_tblidx = nc.dram_tensor("dbg_tblidx", [16, 128], F32,
                                    kind="ExternalOutput")

    with tile.TileContext(nc) as tc:
        with tc.tile_pool(name="sbs", bufs=1) as sbs, \
             tc.tile_pool(name="sbt", bufs=2) as sbt, \
             tc.tile_pool(name="psb", bufs=4, space="PSUM") as psb, \
             tc.tile_pool(name="pss", bufs=4, space="PSUM") as pss, \
             tc.tile_pool(name="dram", bufs=1, space="DRAM") as dram:

            # ---------- persistent small tiles ----------
            br2_row = sbs.tile([1, E], F32)
            nc.sync.dma_start(br2_row[:], br2[:])
            br2_rep = sbs.tile([128, E], F32)
            nc.gpsimd.partition_broadcast(br2_rep[:], br2_row[:])
            b1_sb = sbs.tile([128, NJT], F32)
            nc.sync.dma_start(b1_sb[:], b1[:])
            b2_row = sbs.tile([1, D], F32)
            nc.sync.dma_start(b2_row[:], b2[:])
            b2_rep = sbs.tile([128, D], F32)
            nc.gpsimd.partition_broadcast(b2_rep[:], b2_row[:])
            ut_sb = sbs.tile([128, 128], F32)
            nc.sync.dma_start(ut_sb[:], UT[:])
            ones_sb = sbs.tile([128, 128], F32)
            nc.vector.memset(ones_sb[:], 1.0)
            iota_sb = sbs.tile([128, E], F32)
            nc.sync.dma_start(iota_sb[:], IOTA8[:])
            tokid_sb = sbs.tile([128, 128], F32)
            nc.sync.dma_start(tokid_sb[:], TOKID[:])
            mask_sb = sbs.tile([E, 128], F32)
            nc.sync.dma_start(mask_sb[:], MASK[:])
            shiftk_sb = sbs.tile([128, 1], F32)
            nc.sync.dma_start(shiftk_sb[:], SHIFTK[:])

            gate_all = sbs.tile([128, NTT], F32)
            gidx_rep = sbs.tile([128, 128], I16)     # dispatch gather idx
            ridx_rep = sbs.tile([128, TS // 16], I16)  # recon gather idx
            zbf = sbs.tile([1, D], BF16)
            nc.vector.memset(zbf[:], 0.0)

            # ---------- DRAM scratch ----------
            myslots_d = dram.tile([TS], F32)
            slots_all_d = dram.tile([T], F32)
            cnt_in_d = dram.tile([1, E], F32)
            cnt_all_d = dram.tile([E, E], F32)
            table_d = dram.tile([C + 1, 64], F32)
            sidx_d = dram.tile([T], I16)
            gidx_d = dram.tile([C], I16)
            ridx_d = dram.tile([TS], I16)
            agin_d = dram.tile([C, D], BF16)
            oe_all_d = dram.tile([T + 1, D], BF16)

            nc.sync.dma_start(oe_all_d[T:T + 1, :], zbf[:])

            # ===================== ROUTER + DISPATCH PREP =====================
            with tc.tile_pool(name="rp", bufs=1) as rp:
                oh_all = rp.tile([128, E, NTT], F32)
                eid_all = rp.tile([128, NTT], F32)
                sown_all = rp.tile([128, NTT], F32)
                slot_all = rp.tile([128, NTT], F32)
                rec_all = rp.tile([128, NTT], F32)

                with tc.tile_pool(name="rt", bufs=1) as rt:
                    xt_sb = rt.tile([128, E, TS], F32)
                    nc.sync.dma_start(xt_sb[:], xT[:])
                    wr1_sb = rt.tile([128, E, D], F32)
                    nc.sync.dma_start(wr1_sb[:], wr1[:])
                    wr2_sb = rt.tile([128, E, E], F32)
                    nc.sync.dma_start(wr2_sb[:], wr2[:])
                    br1_sb = rt.tile([128, E], F32)
                    nc.sync.dma_start(br1_sb[:], br1[:])
                    ht_sb = rt.tile([128, E, TS], F32)

                    # hT = relu(wr1.T @ x + br1)   (fp32)
                    for jt in range(E):
                        for tcn in range(TS // 512):
                            ps = psb.tile([128, 512], F32, tag="pbig")
                            for dt in range(E):
                                nc.tensor.matmul(
                                    ps[:],
                                    wr1_sb[:, dt, jt * 128:(jt + 1) * 128],
                                    xt_sb[:, dt, tcn * 512:(tcn + 1) * 512],
                                    start=(dt == 0), stop=(dt == E - 1))
                            nc.scalar.activation(
                                ht_sb[:, jt, tcn * 512:(tcn + 1) * 512], ps[:],
                                Act.Relu, bias=br1_sb[:, jt:jt + 1], scale=1.0)

                    # logits / gate / eid / onehot per 128-token tile
                    for tt in range(NTT):
                        ps = pss.tile([128, E], F32, tag="psmall")
                        for jt in range(E):
                            nc.tensor.matmul(
                                ps[:], ht_sb[:, jt, tt * 128:(tt + 1) * 128],
                                wr2_sb[:, jt, :], start=(jt == 0),
                                stop=(jt == E - 1))
                        lg = sbt.tile([128, E], F32, tag="lg")
                        nc.vector.tensor_tensor(lg[:], ps[:], br2_rep[:], Alu.add)
                        mx = sbt.tile([128, E], F32, tag="mx")
                        mi = sbt.tile([128, E], U32, tag="mi")
                        nc.vector.max_with_indices(mx[:], mi[:], lg[:])
                        nc.vector.tensor_copy(eid_all[:, tt:tt + 1], mi[:, 0:1])
                        negmx = sbt.tile([128, 1], F32, tag="negmx")
                        nc.vector.tensor_scalar(negmx[:], mx[:, 0:1], -1.0, None,
                                                Alu.mult)
                        ex = sbt.tile([128, E], F32, tag="ex")
                        nc.scalar.activation(ex[:], lg[:], Act.Exp,
                                             bias=negmx[:], scale=1.0)
                        sm = sbt.tile([128, 1], F32, tag="sm")
                        nc.vector.reduce_sum(sm[:], ex[:], axis=X)
                        nc.vector.reciprocal(gate_all[:, tt:tt + 1], sm[:])
                        nc.vector.tensor_scalar(
                            oh_all[:, :, tt], iota_sb[:],
                            eid_all[:, tt:tt + 1], None, Alu.is_equal)

                # ---- cumsum within shard (block-triangular matmuls) ----
                for tt in range(NTT):
                    ps = pss.tile([128, E], F32, tag="psmall")
                    nc.tensor.matmul(ps[:], ut_sb[:], oh_all[:, :, tt],
                                     start=True, stop=(tt == 0))
                    for j in range(tt):
                        nc.tensor.matmul(ps[:], ones_sb[:], oh_all[:, :, j],
                                         start=False, stop=(j == tt - 1))
                    tmp = sbt.tile([128, E], F32, tag="tmp8")
                    nc.vector.tensor_tensor(tmp[:], ps[:], oh_all[:, :, tt], Alu.mult)
                    nc.vector.reduce_sum(sown_all[:, tt:tt + 1], tmp[:], axis=X)

                cntps = pss.tile([128, E], F32, tag="psmall")
                for j in range(NTT):
                    nc.tensor.matmul(cntps[:], ones_sb[:], oh_all[:, :, j],
                                     start=(j == 0), stop=(j == NTT - 1))
                cnt_row = sbt.tile([1, E], F32, tag="cntrow")
                nc.vector.tensor_copy(cnt_row[:], cntps[0:1, :])
                nc.sync.dma_start(cnt_in_d[:], cnt_row[:])

                nc.gpsimd.collective_compute(
                    "AllGather", Alu.bypass, replica_groups=RG,
                    ins=[cnt_in_d[:]], outs=[cnt_all_d[:]])

                cnt_sb = sbt.tile([E, E], F32, tag="cntsb")
                nc.sync.dma_start(cnt_sb[:], cnt_all_d[:])
                baseps = pss.tile([128, E], F32, tag="psmall")
                nc.tensor.matmul(baseps[:], mask_sb[:], cnt_sb[:],
                                 start=True, stop=True)
                base_rep = sbt.tile([128, E], F32, tag="baserep")
                nc.vector.tensor_copy(base_rep[:], baseps[:])

                # ---- global slot + recon row per token tile ----
                for tt in range(NTT):
                    tmp = sbt.tile([128, E], F32, tag="tmp8")
                    nc.vector.tensor_tensor(tmp[:], base_rep[:], oh_all[:, :, tt],
                                            Alu.mult)
                    bv = sbt.tile([128, 1], F32, tag="bv")
                    nc.vector.reduce_sum(bv[:], tmp[:], axis=X)
                    pos = sbt.tile([128, 1], F32, tag="pos")
                    nc.vector.tensor_tensor(pos[:], sown_all[:, tt:tt + 1], bv[:],
                                            Alu.add)
                    nc.vector.tensor_scalar(pos[:], pos[:], 1.0, None, Alu.subtract)
                    valid = sbt.tile([128, 1], F32, tag="valid")
                    nc.vector.tensor_scalar(valid[:], pos[:], float(C), None,
                                            Alu.is_lt)
                    sl = sbt.tile([128, 1], F32, tag="sl")
                    nc.vector.tensor_scalar(sl[:], eid_all[:, tt:tt + 1], float(C),
                                            None, Alu.mult)
                    nc.vector.tensor_tensor(sl[:], sl[:], pos[:], Alu.add)
                    nc.vector.tensor_scalar(sl[:], sl[:], float(SENT), None,
                                            Alu.subtract)
                    nc.vector.tensor_tensor(sl[:], sl[:], valid[:], Alu.mult)
                    nc.vector.tensor_scalar(slot_all[:, tt:tt + 1], sl[:],
                                            float(SENT), None, Alu.add)
                    # recon row = (pos>>8)*2048 + eid*256 + (pos&255), or SENT
                    pi = sbt.tile([128, 1], I32, tag="pi")
                    nc.vector.tensor_copy(pi[:], pos[:])
                    lo = sbt.tile([128, 1], I32, tag="lo")
                    nc.vector.tensor_scalar(lo[:], pi[:], CHUNK - 1, None,
                                            Alu.bitwise_and)
                    hi = sbt.tile([128, 1], I32, tag="hi")
                    nc.vector.tensor_scalar(hi[:], pi[:], 8, None,
                                            Alu.arith_shift_right)
                    lof = sbt.tile([128, 1], F32, tag="lof")
                    hif = sbt.tile([128, 1], F32, tag="hif")
                    nc.vector.tensor_copy(lof[:], lo[:])
                    nc.vector.tensor_copy(hif[:], hi[:])
                    rr = sbt.tile([128, 1], F32, tag="rr")
                    nc.vector.tensor_scalar(rr[:], hif[:], float(CHUNK * NC), None,
                                            Alu.mult)
                    tmp2 = sbt.tile([128, 1], F32, tag="tmp2")
                    nc.vector.tensor_scalar(tmp2[:], eid_all[:, tt:tt + 1],
                                            float(CHUNK), None, Alu.mult)
                    nc.vector.tensor_tensor(rr[:], rr[:], tmp2[:], Alu.add)
                    nc.vector.tensor_tensor(rr[:], rr[:], lof[:], Alu.add)
                    nc.vector.tensor_scalar(rr[:], rr[:], float(SENT), None,
                                            Alu.subtract)
                    nc.vector.tensor_tensor(rr[:], rr[:], valid[:], Alu.mult)
                    nc.vector.tensor_scalar(rec_all[:, tt:tt + 1], rr[:],
                                            float(SENT), None, Alu.add)

                if debug_outputs:
                    nc.sync.dma_start(dbg_slot[:], slot_all[:])
                    nc.sync.dma_start(dbg_gate[:], gate_all[:])
                    nc.sync.dma_start(dbg_eid[:], eid_all[:])
                    nc.sync.dma_start(dbg_rec[:], rec_all[:])
                    nc.sync.dma_start(dbg_cnt[:], cnt_sb[:])

                # ---- AllGather slots ----
                nc.sync.dma_start(
                    myslots_d[:].rearrange("(t p) -> p t", p=128), slot_all[:])
                nc.gpsimd.collective_compute(
                    "AllGather", Alu.bypass, replica_groups=RG,
                    ins=[myslots_d[:]], outs=[slots_all_d[:]])

                # ---- recon idx bounce (wrap to gather layout) ----
                ridx16 = sbt.tile([128, NTT], I16, tag="ridx16")
                nc.vector.tensor_copy(ridx16[:], rec_all[:])
                nc.sync.dma_start(
                    ridx_d[:].rearrange("(t p) -> p t", p=128), ridx16[:])
                for rep in range(8):
                    nc.sync.dma_start(
                        ridx_rep[16 * rep:16 * (rep + 1), :],
                        ridx_d[:].rearrange("(c q) -> q c", q=16))

                # ---- table build ----
                with tc.tile_pool(name="tp", bufs=1) as tp:
                    sall = tp.tile([16, T // 16], F32)
                    nc.sync.dma_start(
                        sall[:], slots_all_d[:].rearrange("(c q) -> q c", q=16))
                    nc.vector.tensor_scalar(sall[:], sall[:], shiftk_sb[0:16, :],
                                            None, Alu.subtract)
                    ge0 = tp.tile([16, T // 16], F32)
                    nc.vector.tensor_scalar(ge0[:], sall[:], 0.0, None, Alu.is_ge)
                    ltc = tp.tile([16, T // 16], F32)
                    nc.vector.tensor_scalar(ltc[:], sall[:], float(C), None,
                                            Alu.is_lt)
                    nc.vector.tensor_tensor(ge0[:], ge0[:], ltc[:], Alu.mult)
                    nc.vector.tensor_scalar(sall[:], sall[:], float(C), None,
                                            Alu.subtract)
                    nc.vector.tensor_tensor(sall[:], sall[:], ge0[:], Alu.mult)
                    nc.vector.tensor_scalar(sall[:], sall[:], float(C), None,
                                            Alu.add)
                    sidx16 = tp.tile([16, T // 16], I16)
                    nc.vector.tensor_copy(sidx16[:], sall[:])
                    nc.sync.dma_start(
                        sidx_d[:].rearrange("(c q) -> q c", q=16), sidx16[:])
                    sidx_rep = tp.tile([128, T // 16], I16)
                    for rep in range(8):
                        nc.sync.dma_start(
                            sidx_rep[16 * rep:16 * (rep + 1), :],
                            sidx_d[:].rearrange("(c q) -> q c", q=16))

                    zt = tp.tile([128, 1024], F32)
                    nc.vector.memset(zt[:], 0.0)
                    nc.sync.dma_start(
                        table_d[0:C, :].rearrange("(n p) e -> p n e", p=128),
                        zt[:])
                    nc.sync.dma_start(table_d[C:C + 1, :], zt[0:1, 0:64])

                    pay = tp.tile([128, 128, 64], F32)
                    nc.vector.memset(pay[:], 0.0)
                    nc.vector.tensor_copy(pay[:, :, 0], tokid_sb[:])
                    # SWDGE ring holds 128 descriptors (~16 tokens each), so
                    # split the 16K-token scatter into 1024-token calls.
                    for i in range(T // 1024):
                        nc.gpsimd.dma_scatter_add(
                            table_d[:], pay[:, i * 8:(i + 1) * 8, :],
                            sidx_rep[:, i * 64:(i + 1) * 64], 1024, 1024, 64)

                    tbl = tp.tile([16, 128, 64], F32)
                    nc.sync.dma_start(
                        tbl[:], table_d[0:C, :].rearrange("(c q) e -> q c e", q=16))
                    tid = tp.tile([16, 128], F32)
                    nc.vector.tensor_copy(tid[:], tbl[:, :, 0])
                    emp = tp.tile([16, 128], F32)
                    nc.vector.tensor_scalar(emp[:], tid[:], 0.0, float(SENT + 1),
                                            Alu.is_equal, Alu.mult)
                    nc.vector.tensor_tensor(tid[:], tid[:], emp[:], Alu.add)
                    nc.vector.tensor_scalar(tid[:], tid[:], 1.0, None, Alu.subtract)
                    if debug_outputs:
                        nc.sync.dma_start(dbg_tblidx[:], tid[:])
                    gidx16 = tp.tile([16, 128], I16)
                    nc.vector.tensor_copy(gidx16[:], tid[:])
                    nc.sync.dma_start(
                        gidx_d[:].rearrange("(c q) -> q c", q=16), gidx16[:])
                    for rep in range(8):
                        nc.sync.dma_start(
                            gidx_rep[16 * rep:16 * (rep + 1), :],
                            gidx_d[:].rearrange("(c q) -> q c", q=16))

            # ===================== EXPERT FFN =====================
            with tc.tile_pool(name="fw", bufs=1) as fw, \
                 tc.tile_pool(name="fc", bufs=2) as fc:
                w1_sb = fw.tile([128, E, H], BF16)
                nc.sync.dma_start(w1_sb[:], w1[:])
                w2_sb = fw.tile([128, NJT, D], BF16)
                nc.sync.dma_start(w2_sb[:], w2[:])

                for ch in range(NCH):
                    xet = fc.tile([128, E, CHUNK], BF16, tag="xet")
                    nc.gpsimd.dma_gather(
                        xet[:], xbf[:], gidx_rep[:, ch * 16:(ch + 1) * 16],
                        CHUNK, CHUNK, D, transpose=True)

                    htf = fc.tile([128, NJT, CHUNK], BF16, tag="htf", bufs=1)
                    for jt in range(NJT):
                        ps = psb.tile([128, CHUNK], F32, tag="pbig")
                        for dt in range(E):
                            nc.tensor.matmul(
                                ps[:], w1_sb[:, dt, jt * 128:(jt + 1) * 128],
                                xet[:, dt, :], start=(dt == 0), stop=(dt == E - 1))
                        nc.scalar.activation(htf[:, jt, :], ps[:], Act.Relu,
                                             bias=b1_sb[:, jt:jt + 1], scale=1.0)

                    for ct in range(CHUNK // 128):
                        oe = fc.tile([128, D], BF16, tag="oe")
                        for nt in range(D // 512):
                            ps = psb.tile([128, 512], F32, tag="pbig")
                            for jt in range(NJT):
                                nc.tensor.matmul(
                                    ps[:],
                                    htf[:, jt, ct * 128:(ct + 1) * 128],
                                    w2_sb[:, jt, nt * 512:(nt + 1) * 512],
                                    start=(jt == 0), stop=(jt == NJT - 1))
                            nc.vector.tensor_tensor(
                                oe[:, nt * 512:(nt + 1) * 512], ps[:],
                                b2_rep[:, nt * 512:(nt + 1) * 512], Alu.add)
                        row0 = ch * CHUNK + ct * 128
                        nc.sync.dma_start(agin_d[row0:row0 + 128, :], oe[:])

                    nc.gpsimd.collective_compute(
                        "AllGather", Alu.bypass, replica_groups=RG,
                        ins=[agin_d[ch * CHUNK:(ch + 1) * CHUNK, :]],
                        outs=[oe_all_d[ch * CHUNK * NC:(ch + 1) * CHUNK * NC, :]])

                # ---------- reconstruct my token shard (4 quarters) ----------
                for q in range(4):
                    rec = fc.tile([128, 4, D], BF16, tag="rec")
                    nc.gpsimd.dma_gather(
                        rec[:], oe_all_d[:], ridx_rep[:, q * 32:(q + 1) * 32],
                        512, 512, D, transpose=False)
                    for i in range(4):
                        tt = q * 4 + i
                        of = fc.tile([128, D], F32, tag="of")
                        nc.vector.tensor_scalar(of[:], rec[:, i, :],
                                                gate_all[:, tt:tt + 1], None,
                                                Alu.mult)
                        nc.sync.dma_start(out[tt * 128:(tt + 1) * 128, :], of[:])

    nc.compile()
    return nc


# ---------------------------------------------------------------------------
# host side
# ---------------------------------------------------------------------------
def _to_bf16(a: np.ndarray) -> np.ndarray:
    import jax
    import jax.numpy as jnp
    with jax.default_device(jax.devices("cpu")[0]):
        return np.asarray(jnp.asarray(a, jnp.bfloat16))


_NC_CACHE = {}


def _get_nc(debug_outputs=DEBUG_OUTPUTS):
    if debug_outputs not in _NC_CACHE:
        _NC_CACHE[debug_outputs] = build(debug_outputs)
    return _NC_CACHE[debug_outputs]


def prepare_in_maps(x, wr1, br1, wr2, br2, w1, b1, w2, b2):
    x = np.asarray(x, np.float32)
    wr1 = np.asarray(wr1, np.float32)
    wr2 = np.asarray(wr2, np.float32)
    br1 = np.asarray(br1, np.float32)
    br2 = np.asarray(br2, np.float32)
    w1 = np.asarray(w1, np.float32)
    w2 = np.asarray(w2, np.float32)
    b1 = np.asarray(b1, np.float32)
    b2 = np.asarray(b2, np.float32)

    xpad = np.zeros((T + 1, D), np.float32)
    xpad[:T] = x
    xbf = _to_bf16(xpad)

    iota8 = np.tile(np.arange(E, dtype=np.float32), (128, 1))
    tokid = np.zeros((128, 128), np.float32)
    gi = np.arange(T)
    tokid[gi % 128, gi // 128] = gi.astype(np.float32) + 1.0
    ut = np.triu(np.ones((128, 128), np.float32))

    base = dict(
        wr1=np.ascontiguousarray(wr1.reshape(E, 128, D).transpose(1, 0, 2)),
        wr2=np.ascontiguousarray(wr2.reshape(E, 128, E).transpose(1, 0, 2)),
        br1=np.ascontiguousarray(br1.reshape(E, 128).T),
        br2=br2.reshape(1, E),
        xbf=xbf, UT=ut, IOTA8=iota8, TOKID=tokid,
    )
    maps = []
    for k in range(NC):
        m = dict(base)
        xs = x[k * TS:(k + 1) * TS]                      # [2048, 1024]
        m["xT"] = np.ascontiguousarray(
            xs.T.reshape(E, 128, TS).transpose(1, 0, 2))
        m["w1"] = _to_bf16(np.ascontiguousarray(
            w1[k].reshape(E, 128, H).transpose(1, 0, 2)))
        m["w2"] = _to_bf16(np.ascontiguousarray(
            w2[k].reshape(NJT, 128, D).transpose(1, 0, 2)))
        m["b1"] = np.ascontiguousarray(b1[k].reshape(NJT, 128).T)
        m["b2"] = b2[k].reshape(1, D)
        mask = np.zeros((E, 128), np.float32)
        mask[:k, :] = 1.0
        m["MASK"] = mask
        m["SHIFTK"] = np.full((128, 1), k * C, np.float32)
        maps.append(m)
    return maps


def run(inputs, trace=False, debug_outputs=DEBUG_OUTPUTS, **kw):
    nc = _get_nc(debug_outputs)
    in_maps = prepare_in_maps(**inputs)
    return bass_utils.run_bass_kernel_spmd(
        nc, in_maps, core_ids=list(range(NC)), trace=trace, **kw)


def kernel(**inputs) -> np.ndarray:
    res = run(inputs)
    return np.concatenate([res.results[k]["out"] for k in range(NC)], axis=0)


# revision 8
# speedup vs baseline: 2.5349x; 2.5349x over previous
"""Expert-parallel Switch-Transformer MoE layer for 8 Trainium2 NeuronCores.

Strategy (SPMD, one program, per-core inputs):
  - Token-parallel router: core k routes tokens [k*2048,(k+1)*2048) in fp32
    (argmax must match the fp32 reference; bf16 flips ~31 tokens).
  - Block-triangular-matmul cumsum gives per-shard expert positions; an
    AllGather of per-shard expert counts gives global first-come positions.
  - Global slot = eid*C + pos; slots AllGathered; each core builds its own
    expert's slot->token table with a 256B-row DMA scatter.
  - Expert-parallel FFN (bf16): core k holds expert k's weights; tokens are
    fetched by transpose-dma_gather from a replicated bf16 copy of x.
  - Combine: per-chunk AllGather of expert outputs; each core gathers its own
    tokens' rows by slot, applies its locally-known fp32 gate, writes its
    token shard. Host concatenates the 8 shards.
"""
import sys

for _p in ("/opt/trn_rl_repo", "/root/.axon_site/_ro/trn_rl_repo"):
    if _p not in sys.path:
        sys.path.append(_p)

import numpy as np

import concourse.bacc as bacc
import concourse.bass as bass
import concourse.mybir as mybir
import concourse.tile as tile
from concourse import bass_utils

F32 = mybir.dt.float32
BF16 = mybir.dt.bfloat16
I16 = mybir.dt.int16
I32 = mybir.dt.int32
U32 = mybir.dt.uint32
Alu = mybir.AluOpType
Act = mybir.ActivationFunctionType
X = mybir.AxisListType.X

T, D, E, H = 16384, 1024, 8, 4096
NC = 8
TS = T // NC                # tokens per shard = 2048
C = T // E                  # expert capacity = 2048
NTT = TS // 128             # 16 token tiles per shard
CHUNK = 256                 # FFN token chunk
NCH = C // CHUNK            # 8 chunks
NJT = H // 128              # 32
SENT = T                    # sentinel row id -> zero pad row
RG = [list(range(NC))]

DEBUG_OUTPUTS = False


def build(debug_outputs=DEBUG_OUTPUTS):
    nc = bacc.Bacc("TRN2", target_bir_lowering=False, debug=False, num_devices=NC)

    # --------- per-core inputs (host pre-arranged, straight DMA loads) ---------
    xT = nc.dram_tensor("xT", [128, E, TS], F32, kind="ExternalInput")
    wr1 = nc.dram_tensor("wr1", [128, E, D], F32, kind="ExternalInput")
    wr2 = nc.dram_tensor("wr2", [128, E, E], F32, kind="ExternalInput")
    br1 = nc.dram_tensor("br1", [128, E], F32, kind="ExternalInput")
    br2 = nc.dram_tensor("br2", [1, E], F32, kind="ExternalInput")
    xbf = nc.dram_tensor("xbf", [T + 1, D], BF16, kind="ExternalInput")
    w1 = nc.dram_tensor("w1", [128, E, H], BF16, kind="ExternalInput")
    w2 = nc.dram_tensor("w2", [128, NJT, D], BF16, kind="ExternalInput")
    b1 = nc.dram_tensor("b1", [128, NJT], F32, kind="ExternalInput")
    b2 = nc.dram_tensor("b2", [1, D], F32, kind="ExternalInput")
    UT = nc.dram_tensor("UT", [128, 128], F32, kind="ExternalInput")
    IOTA8 = nc.dram_tensor("IOTA8", [128, E], F32, kind="ExternalInput")
    TOKID = nc.dram_tensor("TOKID", [128, NTT], F32, kind="ExternalInput")
    MASK = nc.dram_tensor("MASK", [E, 128], F32, kind="ExternalInput")
    IDN = nc.dram_tensor("IDN", [128, 128], F32, kind="ExternalInput")

    out = nc.dram_tensor("out", [TS, D], F32, kind="ExternalOutput")
    if debug_outputs:
        dbg_slot = nc.dram_tensor("dbg_slot", [128, NTT], F32, kind="ExternalOutput")
        dbg_gate = nc.dram_tensor("dbg_gate", [128, NTT], F32, kind="ExternalOutput")
        dbg_eid = nc.dram_tensor("dbg_eid", [128, NTT], F32, kind="ExternalOutput")
        dbg_rec = nc.dram_tensor("dbg_rec", [128, NTT], F32, kind="ExternalOutput")
        dbg_cnt = nc.dram_tensor("dbg_cnt", [E, E], F32, kind="ExternalOutput")
        dbg_tblidx = nc.dram_tensor("dbg_tblidx", [128, 16], F32,
                                    kind="ExternalOutput")

    with tile.TileContext(nc) as tc:
        with tc.tile_pool(name="sbs", bufs=1) as sbs, \
             tc.tile_pool(name="sbt", bufs=2) as sbt, \
             tc.tile_pool(name="psb", bufs=4, space="PSUM") as psb, \
             tc.tile_pool(name="pss", bufs=2, space="PSUM") as pss, \
             tc.tile_pool(name="dram", bufs=1, space="DRAM") as dram:

            # ---------- persistent small tiles ----------
            br2_row = sbs.tile([1, E], F32)
            nc.sync.dma_start(br2_row[:], br2[:])
            br2_rep = sbs.tile([128, E], F32)
            nc.gpsimd.partition_broadcast(br2_rep[:], br2_row[:])
            b1_sb = sbs.tile([128, NJT], F32)
            nc.sync.dma_start(b1_sb[:], b1[:])
            b2_row = sbs.tile([1, D], F32)
            nc.sync.dma_start(b2_row[:], b2[:])
            b2_rep = sbs.tile([128, D], F32)
            nc.gpsimd.partition_broadcast(b2_rep[:], b2_row[:])
            ut_sb = sbs.tile([128, 128], F32)
            nc.sync.dma_start(ut_sb[:], UT[:])
            ones_sb = sbs.tile([128, 128], F32)
            nc.vector.memset(ones_sb[:], 1.0)
            iota_sb = sbs.tile([128, E], F32)
            nc.sync.dma_start(iota_sb[:], IOTA8[:])
            tokid_sb = sbs.tile([128, NTT], F32)
            nc.sync.dma_start(tokid_sb[:], TOKID[:])
            mask_sb = sbs.tile([E, 128], F32)
            nc.sync.dma_start(mask_sb[:], MASK[:])
            idn_sb = sbs.tile([128, 128], F32)
            nc.sync.dma_start(idn_sb[:], IDN[:])

            gate_all = sbs.tile([128, NTT], F32)
            gidx_rep = sbs.tile([128, 128], I16)     # dispatch gather idx
            ridx_rep = sbs.tile([128, TS // 16], I16)  # recon gather idx
            zbf = sbs.tile([1, D], BF16)
            nc.vector.memset(zbf[:], 0.0)

            # ---------- DRAM scratch ----------
            cnt_in_d = dram.tile([1, E], F32)
            cnt_all_d = dram.tile([E, E], F32)
            table_d = dram.tile([T + 128, 1], F32)   # full slot->tok+1 table
            mytbl_d = dram.tile([C, 1], F32)         # RS output (my expert)
            ridx_f_d = dram.tile([TS], F32)          # recon idx bounce
            agin_d = dram.tile([C, D], BF16)
            oe_all_d = dram.tile([T + 1, D], BF16)

            nc.sync.dma_start(oe_all_d[T:T + 1, :], zbf[:])

            # ===================== ROUTER + DISPATCH PREP =====================
            with tc.tile_pool(name="rp", bufs=1) as rp:
                oh_all = rp.tile([128, E, NTT], F32)
                eid_all = rp.tile([128, NTT], F32)
                sown_all = rp.tile([128, NTT], F32)
                slot_all = rp.tile([128, NTT], F32)
                rec_all = rp.tile([128, NTT], F32)

                with tc.tile_pool(name="rt", bufs=1) as rt:
                    xt_sb = rt.tile([128, E, TS], F32)
                    nc.sync.dma_start(xt_sb[:], xT[:])
                    wr1_sb = rt.tile([128, E, D], F32)
                    nc.sync.dma_start(wr1_sb[:], wr1[:])
                    wr2_sb = rt.tile([128, E, E], F32)
                    nc.sync.dma_start(wr2_sb[:], wr2[:])
                    br1_sb = rt.tile([128, E], F32)
                    nc.sync.dma_start(br1_sb[:], br1[:])
                    ht_sb = rt.tile([128, E, TS], F32)

                    # hT = relu(wr1.T @ x + br1)   (fp32)
                    for jt in range(E):
                        for tcn in range(TS // 512):
                            ps = psb.tile([128, 512], F32, tag="pbig")
                            for dt in range(E):
                                nc.tensor.matmul(
                                    ps[:],
                                    wr1_sb[:, dt, jt * 128:(jt + 1) * 128],
                                    xt_sb[:, dt, tcn * 512:(tcn + 1) * 512],
                                    start=(dt == 0), stop=(dt == E - 1))
                            nc.scalar.activation(
                                ht_sb[:, jt, tcn * 512:(tcn + 1) * 512], ps[:],
                                Act.Relu, bias=br1_sb[:, jt:jt + 1], scale=1.0)

                    # logits / gate / eid / onehot per 128-token tile
                    for tt in range(NTT):
                        ps = pss.tile([128, E], F32, tag="psmall")
                        for jt in range(E):
                            nc.tensor.matmul(
                                ps[:], ht_sb[:, jt, tt * 128:(tt + 1) * 128],
                                wr2_sb[:, jt, :], start=(jt == 0),
                                stop=(jt == E - 1))
                        lg = sbt.tile([128, E], F32, tag="lg")
                        nc.vector.tensor_tensor(lg[:], ps[:], br2_rep[:], Alu.add)
                        mx = sbt.tile([128, E], F32, tag="mx")
                        mi = sbt.tile([128, E], U32, tag="mi")
                        nc.vector.max_with_indices(mx[:], mi[:], lg[:])
                        nc.vector.tensor_copy(eid_all[:, tt:tt + 1], mi[:, 0:1])
                        negmx = sbt.tile([128, 1], F32, tag="negmx")
                        nc.vector.tensor_scalar(negmx[:], mx[:, 0:1], -1.0, None,
                                                Alu.mult)
                        ex = sbt.tile([128, E], F32, tag="ex")
                        nc.scalar.activation(ex[:], lg[:], Act.Exp,
                                             bias=negmx[:], scale=1.0)
                        sm = sbt.tile([128, 1], F32, tag="sm")
                        nc.vector.reduce_sum(sm[:], ex[:], axis=X)
                        nc.vector.reciprocal(gate_all[:, tt:tt + 1], sm[:])
                        nc.vector.tensor_scalar(
                            oh_all[:, :, tt], iota_sb[:],
                            eid_all[:, tt:tt + 1], None, Alu.is_equal)

                # ---- cumsum within shard (block-triangular matmuls) ----
                for tt in range(NTT):
                    ps = pss.tile([128, E], F32, tag="psmall")
                    nc.tensor.matmul(ps[:], ut_sb[:], oh_all[:, :, tt],
                                     start=True, stop=(tt == 0))
                    for j in range(tt):
                        nc.tensor.matmul(ps[:], ones_sb[:], oh_all[:, :, j],
                                         start=False, stop=(j == tt - 1))
                    tmp = sbt.tile([128, E], F32, tag="tmp8")
                    nc.vector.tensor_tensor(tmp[:], ps[:], oh_all[:, :, tt], Alu.mult)
                    nc.vector.reduce_sum(sown_all[:, tt:tt + 1], tmp[:], axis=X)

                cntps = pss.tile([128, E], F32, tag="psmall")
                for j in range(NTT):
                    nc.tensor.matmul(cntps[:], ones_sb[:], oh_all[:, :, j],
                                     start=(j == 0), stop=(j == NTT - 1))
                cnt_row = sbt.tile([1, E], F32, tag="cntrow")
                nc.vector.tensor_copy(cnt_row[:], cntps[0:1, :])
                nc.sync.dma_start(cnt_in_d[:], cnt_row[:])

                nc.gpsimd.collective_compute(
                    "AllGather", Alu.bypass, replica_groups=RG,
                    ins=[cnt_in_d[:]], outs=[cnt_all_d[:]])

                cnt_sb = sbt.tile([E, E], F32, tag="cntsb")
                nc.sync.dma_start(cnt_sb[:], cnt_all_d[:])
                baseps = pss.tile([128, E], F32, tag="psmall")
                nc.tensor.matmul(baseps[:], mask_sb[:], cnt_sb[:],
                                 start=True, stop=True)
                base_rep = sbt.tile([128, E], F32, tag="baserep")
                nc.vector.tensor_copy(base_rep[:], baseps[:])

                # ---- global slot + recon row per token tile ----
                for tt in range(NTT):
                    tmp = sbt.tile([128, E], F32, tag="tmp8")
                    nc.vector.tensor_tensor(tmp[:], base_rep[:], oh_all[:, :, tt],
                                            Alu.mult)
                    bv = sbt.tile([128, 1], F32, tag="bv")
                    nc.vector.reduce_sum(bv[:], tmp[:], axis=X)
                    pos = sbt.tile([128, 1], F32, tag="pos")
                    nc.vector.tensor_tensor(pos[:], sown_all[:, tt:tt + 1], bv[:],
                                            Alu.add)
                    nc.vector.tensor_scalar(pos[:], pos[:], 1.0, None, Alu.subtract)
                    valid = sbt.tile([128, 1], F32, tag="valid")
                    nc.vector.tensor_scalar(valid[:], pos[:], float(C), None,
                                            Alu.is_lt)
                    sl = sbt.tile([128, 1], F32, tag="sl")
                    nc.vector.tensor_scalar(sl[:], eid_all[:, tt:tt + 1], float(C),
                                            None, Alu.mult)
                    nc.vector.tensor_tensor(sl[:], sl[:], pos[:], Alu.add)
                    nc.vector.tensor_scalar(sl[:], sl[:], float(SENT), None,
                                            Alu.subtract)
                    nc.vector.tensor_tensor(sl[:], sl[:], valid[:], Alu.mult)
                    nc.vector.tensor_scalar(slot_all[:, tt:tt + 1], sl[:],
                                            float(SENT), None, Alu.add)
                    # recon row = (pos>>8)*2048 + eid*256 + (pos&255), or SENT
                    pi = sbt.tile([128, 1], I32, tag="pi")
                    nc.vector.tensor_copy(pi[:], pos[:])
                    lo = sbt.tile([128, 1], I32, tag="lo")
                    nc.vector.tensor_scalar(lo[:], pi[:], CHUNK - 1, None,
                                            Alu.bitwise_and)
                    hi = sbt.tile([128, 1], I32, tag="hi")
                    nc.vector.tensor_scalar(hi[:], pi[:], 8, None,
                                            Alu.arith_shift_right)
                    lof = sbt.tile([128, 1], F32, tag="lof")
                    hif = sbt.tile([128, 1], F32, tag="hif")
                    nc.vector.tensor_copy(lof[:], lo[:])
                    nc.vector.tensor_copy(hif[:], hi[:])
                    rr = sbt.tile([128, 1], F32, tag="rr")
                    nc.vector.tensor_scalar(rr[:], hif[:], float(CHUNK * NC), None,
                                            Alu.mult)
                    tmp2 = sbt.tile([128, 1], F32, tag="tmp2")
                    nc.vector.tensor_scalar(tmp2[:], eid_all[:, tt:tt + 1],
                                            float(CHUNK), None, Alu.mult)
                    nc.vector.tensor_tensor(rr[:], rr[:], tmp2[:], Alu.add)
                    nc.vector.tensor_tensor(rr[:], rr[:], lof[:], Alu.add)
                    nc.vector.tensor_scalar(rr[:], rr[:], float(SENT), None,
                                            Alu.subtract)
                    nc.vector.tensor_tensor(rr[:], rr[:], valid[:], Alu.mult)
                    nc.vector.tensor_scalar(rec_all[:, tt:tt + 1], rr[:],
                                            float(SENT), None, Alu.add)

                if debug_outputs:
                    nc.sync.dma_start(dbg_slot[:], slot_all[:])
                    nc.sync.dma_start(dbg_gate[:], gate_all[:])
                    nc.sync.dma_start(dbg_eid[:], eid_all[:])
                    nc.sync.dma_start(dbg_rec[:], rec_all[:])
                    nc.sync.dma_start(dbg_cnt[:], cnt_sb[:])

                # ---- local scatter of my tokens into the full table ----
                # (hardware-DGE indirect DMA: 2048 x 4B writes), then
                # ReduceScatter(add) delivers each core its expert's slice.
                zt = sbt.tile([128, 129], F32, tag="zt")
                nc.vector.memset(zt[:], 0.0)
                nc.sync.dma_start(
                    table_d[:].rearrange("(p n) e -> p (n e)", p=128), zt[:])
                slotidx = sbt.tile([128, NTT], I32, tag="slotidx")
                nc.vector.tensor_copy(slotidx[:], slot_all[:])
                # HW indirect-DMA semantics: one offset per partition-row,
                # writing that partition's free run contiguously -> use
                # [128, 1] column slices (sim agrees at this shape).
                for i in range(NTT):
                    nc.gpsimd.indirect_dma_start(
                        table_d[:],
                        bass.IndirectOffsetOnAxis(
                            ap=slotidx[:, i:i + 1], axis=0),
                        tokid_sb[:, i:i + 1], None)
                nc.gpsimd.collective_compute(
                    "ReduceScatter", Alu.add, replica_groups=RG,
                    ins=[table_d[0:T, :]], outs=[mytbl_d[:]])

                # ---- readback -> dispatch gather idx (wrapped via PE transp) ----
                tbl_nat = sbt.tile([128, NTT], F32, tag="tblnat")
                nc.sync.dma_start(
                    tbl_nat[:], mytbl_d[:].rearrange("(p n) e -> p (n e)", p=128))
                emp = sbt.tile([128, NTT], F32, tag="emp")
                nc.vector.tensor_scalar(emp[:], tbl_nat[:], 0.0, float(SENT + 1),
                                        Alu.is_equal, Alu.mult)
                nc.vector.tensor_tensor(tbl_nat[:], tbl_nat[:], emp[:], Alu.add)
                nc.vector.tensor_scalar(tbl_nat[:], tbl_nat[:], 1.0, None,
                                        Alu.subtract)
                if debug_outputs:
                    nc.sync.dma_start(dbg_tblidx[:], tbl_nat[:])
                gt_ps = pss.tile([16, 128], F32, tag="ptr")
                nc.tensor.transpose(gt_ps[:], tbl_nat[:], idn_sb[:])
                nc.vector.tensor_copy(gidx_rep[0:16, :], gt_ps[:])
                for rep in range(1, 8):
                    nc.sync.dma_start(gidx_rep[16 * rep:16 * (rep + 1), :],
                                      gidx_rep[0:16, :])

                # ---- recon gather idx (two PE transposes via DRAM bounce) ----
                rt_ps = pss.tile([16, 128], F32, tag="ptr")
                nc.tensor.transpose(rt_ps[:], rec_all[:], idn_sb[:])
                rT = sbt.tile([16, 128], F32, tag="rT")
                nc.vector.tensor_copy(rT[:], rt_ps[:])
                nc.sync.dma_start(
                    ridx_f_d[:].rearrange("(a b) -> a b", a=16), rT[:])
                rn = sbt.tile([128, NTT], F32, tag="rn")
                nc.sync.dma_start(
                    rn[:], ridx_f_d[:].rearrange("(a b) -> a b", a=128))
                rw_ps = pss.tile([16, 128], F32, tag="ptr")
                nc.tensor.transpose(rw_ps[:], rn[:], idn_sb[:])
                nc.vector.tensor_copy(ridx_rep[0:16, :], rw_ps[:])
                for rep in range(1, 8):
                    nc.sync.dma_start(ridx_rep[16 * rep:16 * (rep + 1), :],
                                      ridx_rep[0:16, :])

            # ===================== EXPERT FFN =====================
            with tc.tile_pool(name="fw", bufs=1) as fw, \
                 tc.tile_pool(name="fc", bufs=2) as fc:
                w1_sb = fw.tile([128, E, H], BF16)
                nc.sync.dma_start(w1_sb[:], w1[:])
                w2_sb = fw.tile([128, NJT, D], BF16)
                nc.sync.dma_start(w2_sb[:], w2[:])

                for ch in range(NCH):
                    xet = fc.tile([128, E, CHUNK], BF16, tag="xet")
                    nc.gpsimd.dma_gather(
                        xet[:], xbf[:], gidx_rep[:, ch * 16:(ch + 1) * 16],
                        CHUNK, CHUNK, D, transpose=True)

                    htf = fc.tile([128, NJT, CHUNK], BF16, tag="htf", bufs=1)
                    for jt in range(NJT):
                        ps = psb.tile([128, CHUNK], F32, tag="pbig")
                        for dt in range(E):
                            nc.tensor.matmul(
                                ps[:], w1_sb[:, dt, jt * 128:(jt + 1) * 128],
                                xet[:, dt, :], start=(dt == 0), stop=(dt == E - 1))
                        nc.scalar.activation(htf[:, jt, :], ps[:], Act.Relu,
                                             bias=b1_sb[:, jt:jt + 1], scale=1.0)

                    for ct in range(CHUNK // 128):
                        oe = fc.tile([128, D], BF16, tag="oe")
                        for nt in range(D // 512):
                            ps = psb.tile([128, 512], F32, tag="pbig")
                            for jt in range(NJT):
                                nc.tensor.matmul(
                                    ps[:],
                                    htf[:, jt, ct * 128:(ct + 1) * 128],
                                    w2_sb[:, jt, nt * 512:(nt + 1) * 512],
                                    start=(jt == 0), stop=(jt == NJT - 1))
                            nc.vector.tensor_tensor(
                                oe[:, nt * 512:(nt + 1) * 512], ps[:],
                                b2_rep[:, nt * 512:(nt + 1) * 512], Alu.add)
                        row0 = ch * CHUNK + ct * 128
                        nc.sync.dma_start(agin_d[row0:row0 + 128, :], oe[:])

                    nc.gpsimd.collective_compute(
                        "AllGather", Alu.bypass, replica_groups=RG,
                        ins=[agin_d[ch * CHUNK:(ch + 1) * CHUNK, :]],
                        outs=[oe_all_d[ch * CHUNK * NC:(ch + 1) * CHUNK * NC, :]])

                # ---------- reconstruct my token shard (4 quarters) ----------
                for q in range(4):
                    rec = fc.tile([128, 4, D], BF16, tag="rec")
                    nc.gpsimd.dma_gather(
                        rec[:], oe_all_d[:], ridx_rep[:, q * 32:(q + 1) * 32],
                        512, 512, D, transpose=False)
                    for i in range(4):
                        tt = q * 4 + i
                        of = fc.tile([128, D], F32, tag="of")
                        nc.vector.tensor_scalar(of[:], rec[:, i, :],
                                                gate_all[:, tt:tt + 1], None,
                                                Alu.mult)
                        nc.sync.dma_start(out[tt * 128:(tt + 1) * 128, :], of[:])

    nc.compile()
    return nc


# ---------------------------------------------------------------------------
# host side
# ---------------------------------------------------------------------------
def _to_bf16(a: np.ndarray) -> np.ndarray:
    import jax
    import jax.numpy as jnp
    with jax.default_device(jax.devices("cpu")[0]):
        return np.asarray(jnp.asarray(a, jnp.bfloat16))


_NC_CACHE = {}


def _get_nc(debug_outputs=DEBUG_OUTPUTS):
    if debug_outputs not in _NC_CACHE:
        _NC_CACHE[debug_outputs] = build(debug_outputs)
    return _NC_CACHE[debug_outputs]


def prepare_in_maps(x, wr1, br1, wr2, br2, w1, b1, w2, b2):
    x = np.asarray(x, np.float32)
    wr1 = np.asarray(wr1, np.float32)
    wr2 = np.asarray(wr2, np.float32)
    br1 = np.asarray(br1, np.float32)
    br2 = np.asarray(br2, np.float32)
    w1 = np.asarray(w1, np.float32)
    w2 = np.asarray(w2, np.float32)
    b1 = np.asarray(b1, np.float32)
    b2 = np.asarray(b2, np.float32)

    xpad = np.zeros((T + 1, D), np.float32)
    xpad[:T] = x
    xbf = _to_bf16(xpad)

    iota8 = np.tile(np.arange(E, dtype=np.float32), (128, 1))
    ut = np.triu(np.ones((128, 128), np.float32))
    idn = np.eye(128, dtype=np.float32)

    base = dict(
        wr1=np.ascontiguousarray(wr1.reshape(E, 128, D).transpose(1, 0, 2)),
        wr2=np.ascontiguousarray(wr2.reshape(E, 128, E).transpose(1, 0, 2)),
        br1=np.ascontiguousarray(br1.reshape(E, 128).T),
        br2=br2.reshape(1, E),
        xbf=xbf, UT=ut, IOTA8=iota8, IDN=idn,
    )
    maps = []
    for k in range(NC):
        m = dict(base)
        xs = x[k * TS:(k + 1) * TS]                      # [2048, 1024]
        m["xT"] = np.ascontiguousarray(
            xs.T.reshape(E, 128, TS).transpose(1, 0, 2))
        m["w1"] = _to_bf16(np.ascontiguousarray(
            w1[k].reshape(E, 128, H).transpose(1, 0, 2)))
        m["w2"] = _to_bf16(np.ascontiguousarray(
            w2[k].reshape(NJT, 128, D).transpose(1, 0, 2)))
        m["b1"] = np.ascontiguousarray(b1[k].reshape(NJT, 128).T)
        m["b2"] = b2[k].reshape(1, D)
        mask = np.zeros((E, 128), np.float32)
        mask[:k, :] = 1.0
        m["MASK"] = mask
        tokid = np.zeros((128, NTT), np.float32)
        tl = np.arange(TS)
        tokid[tl % 128, tl // 128] = k * TS + tl + 1.0
        m["TOKID"] = tokid
        maps.append(m)
    return maps


def run(inputs, trace=False, debug_outputs=DEBUG_OUTPUTS, **kw):
    nc = _get_nc(debug_outputs)
    in_maps = prepare_in_maps(**inputs)
    return bass_utils.run_bass_kernel_spmd(
        nc, in_maps, core_ids=list(range(NC)), trace=trace, **kw)


def kernel(**inputs) -> np.ndarray:
    res = run(inputs)
    return np.concatenate([res.results[k]["out"] for k in range(NC)], axis=0)


# revision 10
# speedup vs baseline: 2.6382x; 1.0408x over previous
"""Expert-parallel Switch-Transformer MoE layer for 8 Trainium2 NeuronCores.

Strategy (SPMD, one program, per-core inputs):
  - Token-parallel router: core k routes tokens [k*2048,(k+1)*2048) in fp32
    (argmax must match the fp32 reference; bf16 flips ~31 tokens).
  - Block-triangular-matmul cumsum gives per-shard expert positions; an
    AllGather of per-shard expert counts gives global first-come positions.
  - Global slot = eid*C + pos; slots AllGathered; each core builds its own
    expert's slot->token table with a 256B-row DMA scatter.
  - Expert-parallel FFN (bf16): core k holds expert k's weights; tokens are
    fetched by transpose-dma_gather from a replicated bf16 copy of x.
  - Combine: per-chunk AllGather of expert outputs; each core gathers its own
    tokens' rows by slot, applies its locally-known fp32 gate, writes its
    token shard. Host concatenates the 8 shards.
"""
import sys

for _p in ("/opt/trn_rl_repo", "/root/.axon_site/_ro/trn_rl_repo"):
    if _p not in sys.path:
        sys.path.append(_p)

import numpy as np

import concourse.bacc as bacc
import concourse.bass as bass
import concourse.mybir as mybir
import concourse.tile as tile
from concourse import bass_utils

F32 = mybir.dt.float32
BF16 = mybir.dt.bfloat16
I16 = mybir.dt.int16
I32 = mybir.dt.int32
U32 = mybir.dt.uint32
Alu = mybir.AluOpType
Act = mybir.ActivationFunctionType
X = mybir.AxisListType.X

T, D, E, H = 16384, 1024, 8, 4096
NC = 8
TS = T // NC                # tokens per shard = 2048
C = T // E                  # expert capacity = 2048
NTT = TS // 128             # 16 token tiles per shard
CHUNK = 256                 # FFN token chunk
NCH = C // CHUNK            # 8 chunks
NJT = H // 128              # 32
SENT = T                    # sentinel row id -> zero pad row
RG = [list(range(NC))]

DEBUG_OUTPUTS = False


def build(debug_outputs=DEBUG_OUTPUTS):
    nc = bacc.Bacc("TRN2", target_bir_lowering=False, debug=False, num_devices=NC)

    # --------- per-core inputs (host pre-arranged, straight DMA loads) ---------
    xTh = nc.dram_tensor("xTh", [128, E, TS], BF16, kind="ExternalInput")
    xTl = nc.dram_tensor("xTl", [128, E, TS], BF16, kind="ExternalInput")
    wr1h = nc.dram_tensor("wr1h", [128, E, D], BF16, kind="ExternalInput")
    wr1l = nc.dram_tensor("wr1l", [128, E, D], BF16, kind="ExternalInput")
    wr2 = nc.dram_tensor("wr2", [128, E, E], F32, kind="ExternalInput")
    br1 = nc.dram_tensor("br1", [128, E], F32, kind="ExternalInput")
    br2 = nc.dram_tensor("br2", [1, E], F32, kind="ExternalInput")
    xbf = nc.dram_tensor("xbf", [T + 1, D], BF16, kind="ExternalInput")
    w1 = nc.dram_tensor("w1", [128, E, H], BF16, kind="ExternalInput")
    w2 = nc.dram_tensor("w2", [128, NJT, D], BF16, kind="ExternalInput")
    b1 = nc.dram_tensor("b1", [128, NJT], F32, kind="ExternalInput")
    b2 = nc.dram_tensor("b2", [1, D], F32, kind="ExternalInput")
    UT = nc.dram_tensor("UT", [128, 128], F32, kind="ExternalInput")
    IOTA8 = nc.dram_tensor("IOTA8", [128, E], F32, kind="ExternalInput")
    TOKID = nc.dram_tensor("TOKID", [128, NTT], F32, kind="ExternalInput")
    MASK = nc.dram_tensor("MASK", [E, 128], F32, kind="ExternalInput")
    IDN = nc.dram_tensor("IDN", [128, 128], F32, kind="ExternalInput")

    out = nc.dram_tensor("out", [TS, D], F32, kind="ExternalOutput")
    if debug_outputs:
        dbg_slot = nc.dram_tensor("dbg_slot", [128, NTT], F32, kind="ExternalOutput")
        dbg_gate = nc.dram_tensor("dbg_gate", [128, NTT], F32, kind="ExternalOutput")
        dbg_eid = nc.dram_tensor("dbg_eid", [128, NTT], F32, kind="ExternalOutput")
        dbg_rec = nc.dram_tensor("dbg_rec", [128, NTT], F32, kind="ExternalOutput")
        dbg_cnt = nc.dram_tensor("dbg_cnt", [E, E], F32, kind="ExternalOutput")
        dbg_tblidx = nc.dram_tensor("dbg_tblidx", [128, 16], F32,
                                    kind="ExternalOutput")

    with tile.TileContext(nc) as tc:
        with tc.tile_pool(name="sbs", bufs=1) as sbs, \
             tc.tile_pool(name="sbt", bufs=2) as sbt, \
             tc.tile_pool(name="psb", bufs=4, space="PSUM") as psb, \
             tc.tile_pool(name="pss", bufs=2, space="PSUM") as pss, \
             tc.tile_pool(name="dram", bufs=1, space="DRAM") as dram:

            # ---------- persistent small tiles ----------
            br2_row = sbs.tile([1, E], F32)
            nc.sync.dma_start(br2_row[:], br2[:])
            br2_rep = sbs.tile([128, E], F32)
            nc.gpsimd.partition_broadcast(br2_rep[:], br2_row[:])
            b1_sb = sbs.tile([128, NJT], F32)
            nc.sync.dma_start(b1_sb[:], b1[:])
            b2_row = sbs.tile([1, D], F32)
            nc.sync.dma_start(b2_row[:], b2[:])
            b2_rep = sbs.tile([128, D], F32)
            nc.gpsimd.partition_broadcast(b2_rep[:], b2_row[:])
            ut_sb = sbs.tile([128, 128], F32)
            nc.sync.dma_start(ut_sb[:], UT[:])
            ones_sb = sbs.tile([128, 128], F32)
            nc.vector.memset(ones_sb[:], 1.0)
            iota_sb = sbs.tile([128, E], F32)
            nc.sync.dma_start(iota_sb[:], IOTA8[:])
            tokid_sb = sbs.tile([128, NTT], F32)
            nc.sync.dma_start(tokid_sb[:], TOKID[:])
            mask_sb = sbs.tile([E, 128], F32)
            nc.sync.dma_start(mask_sb[:], MASK[:])
            idn_sb = sbs.tile([128, 128], F32)
            nc.sync.dma_start(idn_sb[:], IDN[:])

            gate_all = sbs.tile([128, NTT], F32)
            gidx_rep = sbs.tile([128, 128], I16)     # dispatch gather idx
            ridx_rep = sbs.tile([128, TS // 16], I16)  # recon gather idx
            zbf = sbs.tile([1, D], BF16)
            nc.vector.memset(zbf[:], 0.0)

            # ---------- DRAM scratch ----------
            cnt_in_d = dram.tile([1, E], F32)
            cnt_all_d = dram.tile([E, E], F32)
            table_d = dram.tile([T + 128], F32)      # full slot->tok+1 table
            mytbl_d = dram.tile([C], F32)            # RS output (my expert)
            ridx_f_d = dram.tile([TS], F32)          # recon idx bounce
            agin_d = dram.tile([C, D], BF16)
            oe_all_d = dram.tile([T + 1, D], BF16)

            nc.sync.dma_start(oe_all_d[T:T + 1, :], zbf[:])

            # ===================== ROUTER + DISPATCH PREP =====================
            with tc.tile_pool(name="rp", bufs=1) as rp:
                oh_all = rp.tile([128, E, NTT], F32)
                eid_all = rp.tile([128, NTT], F32)
                sown_all = rp.tile([128, NTT], F32)
                slot_all = rp.tile([128, NTT], F32)
                rec_all = rp.tile([128, NTT], F32)

                with tc.tile_pool(name="rt", bufs=1) as rt:
                    xth_sb = rt.tile([128, E, TS], BF16)
                    nc.sync.dma_start(xth_sb[:], xTh[:])
                    xtl_sb = rt.tile([128, E, TS], BF16)
                    nc.sync.dma_start(xtl_sb[:], xTl[:])
                    wr1h_sb = rt.tile([128, E, D], BF16)
                    nc.sync.dma_start(wr1h_sb[:], wr1h[:])
                    wr1l_sb = rt.tile([128, E, D], BF16)
                    nc.sync.dma_start(wr1l_sb[:], wr1l[:])
                    wr2_sb = rt.tile([128, E, E], F32)
                    nc.sync.dma_start(wr2_sb[:], wr2[:])
                    br1_sb = rt.tile([128, E], F32)
                    nc.sync.dma_start(br1_sb[:], br1[:])
                    ht_sb = rt.tile([128, E, TS], F32)

                    # hT = relu(wr1.T @ x + br1), 3-term bf16 split
                    # (xh+xl)@(wh+wl) ~ xh@wh + xh@wl + xl@wh; fp32 PSUM
                    # accumulate. Logit err ~1.4e-5 << min top-2 gap.
                    for jt in range(E):
                        for tcn in range(TS // 512):
                            ps = psb.tile([128, 512], F32, tag="pbig")
                            first = True
                            for dt in range(E):
                                js = slice(jt * 128, (jt + 1) * 128)
                                ts_ = slice(tcn * 512, (tcn + 1) * 512)
                                for wop, xop in ((wr1h_sb, xth_sb),
                                                 (wr1l_sb, xth_sb),
                                                 (wr1h_sb, xtl_sb)):
                                    nc.tensor.matmul(
                                        ps[:], wop[:, dt, js], xop[:, dt, ts_],
                                        start=first,
                                        stop=(dt == E - 1 and xop is xtl_sb))
                                    first = False
                            nc.scalar.activation(
                                ht_sb[:, jt, tcn * 512:(tcn + 1) * 512], ps[:],
                                Act.Relu, bias=br1_sb[:, jt:jt + 1], scale=1.0)

                    # logits / gate / eid / onehot per 128-token tile
                    for tt in range(NTT):
                        ps = pss.tile([128, E], F32, tag="psmall")
                        for jt in range(E):
                            nc.tensor.matmul(
                                ps[:], ht_sb[:, jt, tt * 128:(tt + 1) * 128],
                                wr2_sb[:, jt, :], start=(jt == 0),
                                stop=(jt == E - 1))
                        lg = sbt.tile([128, E], F32, tag="lg")
                        nc.vector.tensor_tensor(lg[:], ps[:], br2_rep[:], Alu.add)
                        mx = sbt.tile([128, E], F32, tag="mx")
                        mi = sbt.tile([128, E], U32, tag="mi")
                        nc.vector.max_with_indices(mx[:], mi[:], lg[:])
                        nc.vector.tensor_copy(eid_all[:, tt:tt + 1], mi[:, 0:1])
                        negmx = sbt.tile([128, 1], F32, tag="negmx")
                        nc.vector.tensor_scalar(negmx[:], mx[:, 0:1], -1.0, None,
                                                Alu.mult)
                        ex = sbt.tile([128, E], F32, tag="ex")
                        nc.scalar.activation(ex[:], lg[:], Act.Exp,
                                             bias=negmx[:], scale=1.0)
                        sm = sbt.tile([128, 1], F32, tag="sm")
                        nc.vector.reduce_sum(sm[:], ex[:], axis=X)
                        nc.vector.reciprocal(gate_all[:, tt:tt + 1], sm[:])
                        nc.vector.tensor_scalar(
                            oh_all[:, :, tt], iota_sb[:],
                            eid_all[:, tt:tt + 1], None, Alu.is_equal)

                # ---- cumsum within shard (block-triangular matmuls) ----
                for tt in range(NTT):
                    ps = pss.tile([128, E], F32, tag="psmall")
                    nc.tensor.matmul(ps[:], ut_sb[:], oh_all[:, :, tt],
                                     start=True, stop=(tt == 0))
                    for j in range(tt):
                        nc.tensor.matmul(ps[:], ones_sb[:], oh_all[:, :, j],
                                         start=False, stop=(j == tt - 1))
                    tmp = sbt.tile([128, E], F32, tag="tmp8")
                    nc.vector.tensor_tensor(tmp[:], ps[:], oh_all[:, :, tt], Alu.mult)
                    nc.vector.reduce_sum(sown_all[:, tt:tt + 1], tmp[:], axis=X)

                cntps = pss.tile([128, E], F32, tag="psmall")
                for j in range(NTT):
                    nc.tensor.matmul(cntps[:], ones_sb[:], oh_all[:, :, j],
                                     start=(j == 0), stop=(j == NTT - 1))
                cnt_row = sbt.tile([1, E], F32, tag="cntrow")
                nc.vector.tensor_copy(cnt_row[:], cntps[0:1, :])
                nc.sync.dma_start(cnt_in_d[:], cnt_row[:])

                nc.gpsimd.collective_compute(
                    "AllGather", Alu.bypass, replica_groups=RG,
                    ins=[cnt_in_d[:]], outs=[cnt_all_d[:]])

                cnt_sb = sbt.tile([E, E], F32, tag="cntsb")
                nc.sync.dma_start(cnt_sb[:], cnt_all_d[:])
                baseps = pss.tile([128, E], F32, tag="psmall")
                nc.tensor.matmul(baseps[:], mask_sb[:], cnt_sb[:],
                                 start=True, stop=True)
                base_rep = sbt.tile([128, E], F32, tag="baserep")
                nc.vector.tensor_copy(base_rep[:], baseps[:])

                # ---- global slot + recon row per token tile ----
                for tt in range(NTT):
                    tmp = sbt.tile([128, E], F32, tag="tmp8")
                    nc.vector.tensor_tensor(tmp[:], base_rep[:], oh_all[:, :, tt],
                                            Alu.mult)
                    bv = sbt.tile([128, 1], F32, tag="bv")
                    nc.vector.reduce_sum(bv[:], tmp[:], axis=X)
                    pos = sbt.tile([128, 1], F32, tag="pos")
                    nc.vector.tensor_tensor(pos[:], sown_all[:, tt:tt + 1], bv[:],
                                            Alu.add)
                    nc.vector.tensor_scalar(pos[:], pos[:], 1.0, None, Alu.subtract)
                    valid = sbt.tile([128, 1], F32, tag="valid")
                    nc.vector.tensor_scalar(valid[:], pos[:], float(C), None,
                                            Alu.is_lt)
                    sl = sbt.tile([128, 1], F32, tag="sl")
                    nc.vector.tensor_scalar(sl[:], eid_all[:, tt:tt + 1], float(C),
                                            None, Alu.mult)
                    nc.vector.tensor_tensor(sl[:], sl[:], pos[:], Alu.add)
                    nc.vector.tensor_scalar(sl[:], sl[:], float(SENT), None,
                                            Alu.subtract)
                    nc.vector.tensor_tensor(sl[:], sl[:], valid[:], Alu.mult)
                    nc.vector.tensor_scalar(slot_all[:, tt:tt + 1], sl[:],
                                            float(SENT), None, Alu.add)
                    # recon row = (pos>>8)*2048 + eid*256 + (pos&255), or SENT
                    pi = sbt.tile([128, 1], I32, tag="pi")
                    nc.vector.tensor_copy(pi[:], pos[:])
                    lo = sbt.tile([128, 1], I32, tag="lo")
                    nc.vector.tensor_scalar(lo[:], pi[:], CHUNK - 1, None,
                                            Alu.bitwise_and)
                    hi = sbt.tile([128, 1], I32, tag="hi")
                    nc.vector.tensor_scalar(hi[:], pi[:], 8, None,
                                            Alu.arith_shift_right)
                    lof = sbt.tile([128, 1], F32, tag="lof")
                    hif = sbt.tile([128, 1], F32, tag="hif")
                    nc.vector.tensor_copy(lof[:], lo[:])
                    nc.vector.tensor_copy(hif[:], hi[:])
                    rr = sbt.tile([128, 1], F32, tag="rr")
                    nc.vector.tensor_scalar(rr[:], hif[:], float(CHUNK * NC), None,
                                            Alu.mult)
                    tmp2 = sbt.tile([128, 1], F32, tag="tmp2")
                    nc.vector.tensor_scalar(tmp2[:], eid_all[:, tt:tt + 1],
                                            float(CHUNK), None, Alu.mult)
                    nc.vector.tensor_tensor(rr[:], rr[:], tmp2[:], Alu.add)
                    nc.vector.tensor_tensor(rr[:], rr[:], lof[:], Alu.add)
                    nc.vector.tensor_scalar(rr[:], rr[:], float(SENT), None,
                                            Alu.subtract)
                    nc.vector.tensor_tensor(rr[:], rr[:], valid[:], Alu.mult)
                    nc.vector.tensor_scalar(rec_all[:, tt:tt + 1], rr[:],
                                            float(SENT), None, Alu.add)

                if debug_outputs:
                    nc.sync.dma_start(dbg_slot[:], slot_all[:])
                    nc.sync.dma_start(dbg_gate[:], gate_all[:])
                    nc.sync.dma_start(dbg_eid[:], eid_all[:])
                    nc.sync.dma_start(dbg_rec[:], rec_all[:])
                    nc.sync.dma_start(dbg_cnt[:], cnt_sb[:])

                # ---- local scatter of my tokens into the full table ----
                # (hardware-DGE indirect DMA: 2048 x 4B writes), then
                # ReduceScatter(add) delivers each core its expert's slice.
                zt = sbt.tile([128, 129], F32, tag="zt")
                nc.vector.memset(zt[:], 0.0)
                nc.sync.dma_start(
                    table_d[:].rearrange("(p n) -> p n", p=128), zt[:])
                slotidx = sbt.tile([128, NTT], I32, tag="slotidx")
                nc.vector.tensor_copy(slotidx[:], slot_all[:])
                # HW indirect-DMA semantics: one offset per partition-row,
                # writing that partition's free run contiguously -> use
                # [128, 1] column slices (sim agrees at this shape).
                table2d = table_d[:].rearrange("(n e) -> n e", e=1)
                for i in range(NTT):
                    nc.gpsimd.indirect_dma_start(
                        table2d,
                        bass.IndirectOffsetOnAxis(
                            ap=slotidx[:, i:i + 1], axis=0),
                        tokid_sb[:, i:i + 1], None)
                nc.gpsimd.collective_compute(
                    "ReduceScatter", Alu.add, replica_groups=RG,
                    ins=[table_d[0:T]], outs=[mytbl_d[:]])

                # ---- readback -> dispatch gather idx (wrapped via PE transp) ----
                tbl_nat = sbt.tile([128, NTT], F32, tag="tblnat")
                nc.sync.dma_start(
                    tbl_nat[:], mytbl_d[:].rearrange("(p n) -> p n", p=128))
                emp = sbt.tile([128, NTT], F32, tag="emp")
                nc.vector.tensor_scalar(emp[:], tbl_nat[:], 0.0, float(SENT + 1),
                                        Alu.is_equal, Alu.mult)
                nc.vector.tensor_tensor(tbl_nat[:], tbl_nat[:], emp[:], Alu.add)
                nc.vector.tensor_scalar(tbl_nat[:], tbl_nat[:], 1.0, None,
                                        Alu.subtract)
                if debug_outputs:
                    nc.sync.dma_start(dbg_tblidx[:], tbl_nat[:])
                gt_ps = pss.tile([16, 128], F32, tag="ptr")
                nc.tensor.transpose(gt_ps[:], tbl_nat[:], idn_sb[:])
                nc.vector.tensor_copy(gidx_rep[0:16, :], gt_ps[:])
                for rep in range(1, 8):
                    nc.sync.dma_start(gidx_rep[16 * rep:16 * (rep + 1), :],
                                      gidx_rep[0:16, :])

                # ---- recon gather idx (two PE transposes via DRAM bounce) ----
                rt_ps = pss.tile([16, 128], F32, tag="ptr")
                nc.tensor.transpose(rt_ps[:], rec_all[:], idn_sb[:])
                rT = sbt.tile([16, 128], F32, tag="rT")
                nc.vector.tensor_copy(rT[:], rt_ps[:])
                nc.sync.dma_start(
                    ridx_f_d[:].rearrange("(a b) -> a b", a=16), rT[:])
                rn = sbt.tile([128, NTT], F32, tag="rn")
                nc.sync.dma_start(
                    rn[:], ridx_f_d[:].rearrange("(a b) -> a b", a=128))
                rw_ps = pss.tile([16, 128], F32, tag="ptr")
                nc.tensor.transpose(rw_ps[:], rn[:], idn_sb[:])
                nc.vector.tensor_copy(ridx_rep[0:16, :], rw_ps[:])
                for rep in range(1, 8):
                    nc.sync.dma_start(ridx_rep[16 * rep:16 * (rep + 1), :],
                                      ridx_rep[0:16, :])

            # ===================== EXPERT FFN =====================
            with tc.tile_pool(name="fw", bufs=1) as fw, \
                 tc.tile_pool(name="fc", bufs=2) as fc:
                w1_sb = fw.tile([128, E, H], BF16)
                nc.sync.dma_start(w1_sb[:], w1[:])
                w2_sb = fw.tile([128, NJT, D], BF16)
                nc.sync.dma_start(w2_sb[:], w2[:])

                for ch in range(NCH):
                    xet = fc.tile([128, E, CHUNK], BF16, tag="xet")
                    nc.gpsimd.dma_gather(
                        xet[:], xbf[:], gidx_rep[:, ch * 16:(ch + 1) * 16],
                        CHUNK, CHUNK, D, transpose=True)

                    htf = fc.tile([128, NJT, CHUNK], BF16, tag="htf", bufs=1)
                    for jt in range(NJT):
                        ps = psb.tile([128, CHUNK], F32, tag="pbig")
                        for dt in range(E):
                            nc.tensor.matmul(
                                ps[:], w1_sb[:, dt, jt * 128:(jt + 1) * 128],
                                xet[:, dt, :], start=(dt == 0), stop=(dt == E - 1))
                        nc.scalar.activation(htf[:, jt, :], ps[:], Act.Relu,
                                             bias=b1_sb[:, jt:jt + 1], scale=1.0)

                    for ct in range(CHUNK // 128):
                        oe = fc.tile([128, D], BF16, tag="oe")
                        for nt in range(D // 512):
                            ps = psb.tile([128, 512], F32, tag="pbig")
                            for jt in range(NJT):
                                nc.tensor.matmul(
                                    ps[:],
                                    htf[:, jt, ct * 128:(ct + 1) * 128],
                                    w2_sb[:, jt, nt * 512:(nt + 1) * 512],
                                    start=(jt == 0), stop=(jt == NJT - 1))
                            nc.vector.tensor_tensor(
                                oe[:, nt * 512:(nt + 1) * 512], ps[:],
                                b2_rep[:, nt * 512:(nt + 1) * 512], Alu.add)
                        row0 = ch * CHUNK + ct * 128
                        nc.sync.dma_start(agin_d[row0:row0 + 128, :], oe[:])

                    nc.gpsimd.collective_compute(
                        "AllGather", Alu.bypass, replica_groups=RG,
                        ins=[agin_d[ch * CHUNK:(ch + 1) * CHUNK, :]],
                        outs=[oe_all_d[ch * CHUNK * NC:(ch + 1) * CHUNK * NC, :]])

                # ---------- reconstruct my token shard (4 quarters) ----------
                for q in range(4):
                    rec = fc.tile([128, 4, D], BF16, tag="rec")
                    nc.gpsimd.dma_gather(
                        rec[:], oe_all_d[:], ridx_rep[:, q * 32:(q + 1) * 32],
                        512, 512, D, transpose=False)
                    for i in range(4):
                        tt = q * 4 + i
                        of = fc.tile([128, D], F32, tag="of")
                        nc.vector.tensor_scalar(of[:], rec[:, i, :],
                                                gate_all[:, tt:tt + 1], None,
                                                Alu.mult)
                        nc.sync.dma_start(out[tt * 128:(tt + 1) * 128, :], of[:])

    nc.compile()
    return nc


# ---------------------------------------------------------------------------
# host side
# ---------------------------------------------------------------------------
def _to_bf16(a: np.ndarray) -> np.ndarray:
    import jax
    import jax.numpy as jnp
    with jax.default_device(jax.devices("cpu")[0]):
        return np.asarray(jnp.asarray(a, jnp.bfloat16))


_NC_CACHE = {}


def _get_nc(debug_outputs=DEBUG_OUTPUTS):
    if debug_outputs not in _NC_CACHE:
        _NC_CACHE[debug_outputs] = build(debug_outputs)
    return _NC_CACHE[debug_outputs]


def prepare_in_maps(x, wr1, br1, wr2, br2, w1, b1, w2, b2):
    x = np.asarray(x, np.float32)
    wr1 = np.asarray(wr1, np.float32)
    wr2 = np.asarray(wr2, np.float32)
    br1 = np.asarray(br1, np.float32)
    br2 = np.asarray(br2, np.float32)
    w1 = np.asarray(w1, np.float32)
    w2 = np.asarray(w2, np.float32)
    b1 = np.asarray(b1, np.float32)
    b2 = np.asarray(b2, np.float32)

    xpad = np.zeros((T + 1, D), np.float32)
    xpad[:T] = x
    xbf = _to_bf16(xpad)

    iota8 = np.tile(np.arange(E, dtype=np.float32), (128, 1))
    ut = np.triu(np.ones((128, 128), np.float32))
    idn = np.eye(128, dtype=np.float32)

    wr1p = np.ascontiguousarray(wr1.reshape(E, 128, D).transpose(1, 0, 2))
    wr1h_ = _to_bf16(wr1p)
    base = dict(
        wr1h=wr1h_, wr1l=_to_bf16(wr1p - wr1h_.astype(np.float32)),
        wr2=np.ascontiguousarray(wr2.reshape(E, 128, E).transpose(1, 0, 2)),
        br1=np.ascontiguousarray(br1.reshape(E, 128).T),
        br2=br2.reshape(1, E),
        xbf=xbf, UT=ut, IOTA8=iota8, IDN=idn,
    )
    maps = []
    for k in range(NC):
        m = dict(base)
        xs = x[k * TS:(k + 1) * TS]                      # [2048, 1024]
        xt = np.ascontiguousarray(xs.T.reshape(E, 128, TS).transpose(1, 0, 2))
        xh = _to_bf16(xt)
        m["xTh"] = xh
        m["xTl"] = _to_bf16(xt - xh.astype(np.float32))
        m["w1"] = _to_bf16(np.ascontiguousarray(
            w1[k].reshape(E, 128, H).transpose(1, 0, 2)))
        m["w2"] = _to_bf16(np.ascontiguousarray(
            w2[k].reshape(NJT, 128, D).transpose(1, 0, 2)))
        m["b1"] = np.ascontiguousarray(b1[k].reshape(NJT, 128).T)
        m["b2"] = b2[k].reshape(1, D)
        mask = np.zeros((E, 128), np.float32)
        mask[:k, :] = 1.0
        m["MASK"] = mask
        tokid = np.zeros((128, NTT), np.float32)
        tl = np.arange(TS)
        tokid[tl % 128, tl // 128] = k * TS + tl + 1.0
        m["TOKID"] = tokid
        maps.append(m)
    return maps


def run(inputs, trace=False, debug_outputs=DEBUG_OUTPUTS, **kw):
    nc = _get_nc(debug_outputs)
    in_maps = prepare_in_maps(**inputs)
    return bass_utils.run_bass_kernel_spmd(
        nc, in_maps, core_ids=list(range(NC)), trace=trace, **kw)


def kernel(**inputs) -> np.ndarray:
    res = run(inputs)
    return np.concatenate([res.results[k]["out"] for k in range(NC)], axis=0)
